# revision 8
# baseline (speedup 1.0000x reference)
"""DCGRU cell on 8 Trainium2 NeuronCores — data-parallel over batch.

Problem: nn_DCGRUCell (B=64, N=1024, D_IN=2, U=64, K=2, 2 supports).
Sharding: batch 64 -> 8 cores x 8 local batches (j). Supports + weights
replicated per core; everything else fully local, no collectives.

v3: ZERO per-rep DRAM round-trips. All layout transposes run through the
PE systolic array (SBUF -> PSUM -> SBUF); every tensor stays resident in
SBUF. The only per-rep DMA is the 8-tile output store. (v2 routed
transposes through DRAM DMA-xbar; under this execution environment each
serialized DMA generation costs ~ms, which dominated the runtime.)

Per-core layout:
  hxr[t]   [128, 512] bf16  node-domain hx; cols j*64+u  (slot x0, const)
  hA/hB[t] [128, 512] bf16  diffusion ping-pong (x1..x4 node-domain)
  h0b[t]   [128, 512] bf16  gconv2 x'0 = r*hx node-domain
  hallT[s][p] [128,1024] bf16 feature-domain slots; rows (j%2)*64+u for
           j in {2p, 2p+1}; cols = node
  value[j] [128, 1024] bf16 gconv1: rows 0:64=r^T, 64:128=u^T; gconv2
           tanh overwrites rows 0:64 with c^T
  stgall   [80, 1024] bf16  A-family ^T packed rows m*16+(2j+f)
Projection per (j, c2-half): 5 H-matmuls (k=64) + 1 A-matmul (k=80
against per-j weights wa_*j). Blend in node domain, single out DMA gen.
"""

import numpy as np
import ml_dtypes

import concourse.bass as bass
import concourse.tile as tile
import concourse.mybir as mybir
from concourse import bacc
from concourse.bass_utils import run_bass_kernel_spmd

BF = mybir.dt.bfloat16
F32 = mybir.dt.float32
AF = mybir.ActivationFunctionType
OP = mybir.AluOpType

B, N, D_IN, U = 64, 1024, 2, 64
NCORES, J = 8, 8
NT = 8
O1, O2 = 2 * U, U

_CACHE = {}
STAGE = 99  # build cutoff for profiling components


def _build(reps=1):
    nc = bacc.Bacc(None)

    s0t_d = nc.dram_tensor("s0t", [N, N], BF, kind="ExternalInput")
    s1t_d = nc.dram_tensor("s1t", [N, N], BF, kind="ExternalInput")
    hxr_d = nc.dram_tensor("hxr", [N, J * U], BF, kind="ExternalInput")
    hxrf_d = nc.dram_tensor("hxrf", [N, J * U], F32, kind="ExternalInput")
    a0r_d = nc.dram_tensor("a0r", [N, 16], BF, kind="ExternalInput")
    a0t_d = nc.dram_tensor("a0t", [16, N], BF, kind="ExternalInput")
    wo_g_d = [nc.dram_tensor(f"wo_g{g}", [128, O1], BF, kind="ExternalInput") for g in range(5)]
    wu_g_d = [nc.dram_tensor(f"wu_g{g}", [128, O2], BF, kind="ExternalInput") for g in range(5)]
    waoj_d = nc.dram_tensor("waoj", [80, J * O1], BF, kind="ExternalInput")
    wauj_d = nc.dram_tensor("wauj", [80, J * O2], BF, kind="ExternalInput")
    b_o_d = nc.dram_tensor("b_o", [O1, 1], F32, kind="ExternalInput")
    b_u_d = nc.dram_tensor("b_u", [O2, 1], F32, kind="ExternalInput")
    ident_d = nc.dram_tensor("ident", [128, 128], BF, kind="ExternalInput")
    identb_d = nc.dram_tensor("identb", [128, 64], BF, kind="ExternalInput")
    psel_d = nc.dram_tensor("psel", [16, 400], BF, kind="ExternalInput")
    out_d = nc.dram_tensor("out", [J, N * U], F32, kind="ExternalOutput")

    with tile.TileContext(nc) as tc:
        with (
            tc.tile_pool(name="const", bufs=1) as cp,
            tc.tile_pool(name="state", bufs=1) as hp,
            tc.tile_pool(name="pd", bufs=3, space="PSUM") as pdp,
            tc.tile_pool(name="pt", bufs=3, space="PSUM") as ptp,
            tc.tile_pool(name="pp", bufs=2, space="PSUM") as ppp,
        ):
            env = {}
            # ---- constants ----
            s0t = [cp.tile([128, N], BF, name=f"s0t{k}") for k in range(NT)]
            s1t = [cp.tile([128, N], BF, name=f"s1t{k}") for k in range(NT)]
            for k in range(NT):
                nc.sync.dma_start(s0t[k], s0t_d[k * 128:(k + 1) * 128, :])
                nc.sync.dma_start(s1t[k], s1t_d[k * 128:(k + 1) * 128, :])
            wo_g = [cp.tile([128, O1], BF, name=f"wo_g{g}") for g in range(5)]
            wu_g = [cp.tile([128, O2], BF, name=f"wu_g{g}") for g in range(5)]
            for g in range(5):
                nc.sync.dma_start(wo_g[g], wo_g_d[g][:, :])
                nc.sync.dma_start(wu_g[g], wu_g_d[g][:, :])
            waoj = cp.tile([80, J * O1], BF, name="waoj")
            wauj = cp.tile([80, J * O2], BF, name="wauj")
            nc.sync.dma_start(waoj, waoj_d[:, :])
            nc.sync.dma_start(wauj, wauj_d[:, :])
            b_o = cp.tile([O1, 1], F32, name="b_o")
            b_u = cp.tile([O2, 1], F32, name="b_u")
            nc.sync.dma_start(b_o, b_o_d[:, :])
            nc.sync.dma_start(b_u, b_u_d[:, :])
            ident = cp.tile([128, 128], BF, name="ident")
            identb = cp.tile([128, 64], BF, name="identb")
            psel = cp.tile([16, 400], BF, name="psel")
            nc.sync.dma_start(ident, ident_d[:, :])
            nc.sync.dma_start(identb, identb_d[:, :])
            nc.sync.dma_start(psel, psel_d[:, :])

            hxr = [cp.tile([128, J * U], BF, name=f"hxr{t}") for t in range(NT)]
            hxrf = [cp.tile([128, J * U], F32, name=f"hxrf{t}") for t in range(NT)]
            a0r = [cp.tile([128, 16], BF, name=f"a0r{t}") for t in range(NT)]
            a0t = cp.tile([16, N], BF, name="a0t")
            nc.sync.dma_start(a0t, a0t_d[:, :])
            for t in range(NT):
                nc.sync.dma_start(hxr[t], hxr_d[t * 128:(t + 1) * 128, :])
                nc.sync.dma_start(hxrf[t], hxrf_d[t * 128:(t + 1) * 128, :])
                nc.sync.dma_start(a0r[t], a0r_d[t * 128:(t + 1) * 128, :])

            # ---- persistent working state ----
            hA = [hp.tile([128, J * U], BF, name=f"hA{t}") for t in range(NT)]
            hB = [hp.tile([128, J * U], BF, name=f"hB{t}") for t in range(NT)]
            h0b = [hp.tile([128, J * U], BF, name=f"h0b{t}") for t in range(NT)]
            hallT = [[hp.tile([128, N], BF, name=f"hT{s}_{p}") for p in range(4)]
                     for s in range(5)]
            value = [hp.tile([128, N], BF, name=f"value{j}") for j in range(J)]
            un = [hp.tile([128, J * U], BF, name=f"un{t}") for t in range(NT)]
            cn = [hp.tile([128, J * U], BF, name=f"cn{t}") for t in range(NT)]
            orow = [hp.tile([128, J * U], F32, name=f"orow{t}") for t in range(NT)]
            # A-family (rep-invariant)
            amt = [a0t] + [cp.tile([16, N], BF, name=f"a{m}t") for m in range(1, 5)]
            a1r = [cp.tile([128, 16], BF, name=f"a1r{t}") for t in range(NT)]
            a3r = [cp.tile([128, 16], BF, name=f"a3r{t}") for t in range(NT)]
            stgall = cp.tile([80, N], BF, name="stgall")

            env.update(locals())
            for rep in range(reps):
                _emit_body(env, rep)
    nc.compile()
    return nc


def _emit_body(env, rep):
    nc = env["nc"]
    s0t, s1t = env["s0t"], env["s1t"]
    wo_g, wu_g = env["wo_g"], env["wu_g"]
    waoj, wauj = env["waoj"], env["wauj"]
    b_o, b_u = env["b_o"], env["b_u"]
    ident, identb, psel = env["ident"], env["identb"], env["psel"]
    hxr, hxrf = env["hxr"], env["hxrf"]
    a0r, a0t = env["a0r"], env["a0t"]
    hA, hB, h0b = env["hA"], env["hB"], env["h0b"]
    hallT, value = env["hallT"], env["value"]
    un, cn, orow = env["un"], env["cn"], env["orow"]
    amt, a1r, a3r, stgall = env["amt"], env["a1r"], env["a3r"], env["stgall"]
    pdp, ptp, ppp = env["pdp"], env["ptp"], env["ppp"]
    out_d = env["out_d"]
    R = f"r{rep}"
    uid = [0]

    def nm(pfx):
        uid[0] += 1
        return f"{pfx}{R}_{uid[0]}"

    def step(dst, src, st, base):
        # dst[it] = S @ src  (or 2*S@src - base)  -- node domain
        for it in range(NT):
            pd = pdp.tile([128, J * U], F32, name=nm("pd"), tag="pd")
            for kt in range(NT):
                nc.tensor.matmul(pd, st[kt][:, it * 128:(it + 1) * 128], src[kt],
                                 start=(kt == 0), stop=(kt == NT - 1))
            if base is None:
                nc.vector.tensor_copy(dst[it], pd)
            else:
                nc.vector.scalar_tensor_tensor(
                    out=dst[it], in0=pd, scalar=2.0, in1=base[it],
                    op0=OP.mult, op1=OP.subtract)

    def t_slot(s, src):
        # hallT[s] = src^T  (node [1024, 512] -> feature [512, 1024])
        for p in range(4):
            for half in range(2):
                pt = ptp.tile([128, 512], BF, name=nm("pt"), tag="pt")
                for ti in range(4):
                    t = half * 4 + ti
                    nc.tensor.transpose(pt[:, ti * 128:(ti + 1) * 128],
                                        src[t][:, p * 128:(p + 1) * 128], ident)
                nc.vector.tensor_copy(
                    hallT[s][p][:, half * 512:(half + 1) * 512], pt)

    def t_val(rows, dst, mul_by=None):
        # dst[t][:, j*64:(j+1)*64] = value[j][rows].T  (64-row transposes);
        # with mul_by, dst[t] = transpose * mul_by[t] fused from PSUM.
        lo = rows.start
        for t in range(NT):
            pt = ptp.tile([128, 512], BF, name=nm("pv"), tag="pt")
            for j in range(J):
                nc.tensor.transpose(pt[:, j * 64:(j + 1) * 64],
                                    value[j][rows, t * 128:(t + 1) * 128],
                                    identb[lo:lo + 64, :])
            if mul_by is None:
                nc.vector.tensor_copy(dst[t], pt)
            else:
                nc.vector.tensor_mul(dst[t], pt, mul_by[t])

    def a_family():
        # A-family ^T tiles [16, 1024]: a1,a2 (S0), a3,a4 (S1); pack stgall.
        for st, rowt, i1, i2 in ((s0t, a1r, 1, 2), (s1t, a3r, 3, 4)):
            for c2 in range(2):
                cs = slice(c2 * 512, (c2 + 1) * 512)
                pa = ppp.tile([16, 512], F32, name=nm("pa"), tag="pp")
                for kt in range(NT):
                    nc.tensor.matmul(pa, a0r[kt], st[kt][:, cs],
                                     start=(kt == 0), stop=(kt == NT - 1))
                nc.any.tensor_copy(amt[i1][:, cs], pa)
            for t in range(NT):
                pr = ptp.tile([128, 16], BF, name=nm("pr"), tag="pt")
                nc.tensor.transpose(pr, amt[i1][:, t * 128:(t + 1) * 128],
                                    identb[0:16, 0:16])
                nc.any.tensor_copy(rowt[t], pr)
            for c2 in range(2):
                cs = slice(c2 * 512, (c2 + 1) * 512)
                pa = ppp.tile([16, 512], F32, name=nm("pa2"), tag="pp")
                for kt in range(NT):
                    nc.tensor.matmul(pa, rowt[kt], st[kt][:, cs],
                                     start=(kt == 0), stop=(kt == NT - 1))
                nc.vector.scalar_tensor_tensor(
                    out=amt[i2][:, cs], in0=pa, scalar=2.0, in1=a0t[:, cs],
                    op0=OP.mult, op1=OP.subtract)
        for c2 in range(2):
            cs = slice(c2 * 512, (c2 + 1) * 512)
            pk = ppp.tile([80, 512], F32, name=nm("pk"), tag="pp")
            for m in range(5):
                nc.tensor.matmul(pk, psel[:, m * 80:(m + 1) * 80], amt[m][:, cs],
                                 start=(m == 0), stop=(m == 4))
            nc.any.tensor_copy(stgall[:, cs], pk)

    def project(gc):
        wg, wa, ob, bias = ((wo_g, waoj, O1, b_o) if gc == 0
                            else (wu_g, wauj, O2, b_u))
        for p in range(4):
            for j in (2 * p, 2 * p + 1):
                jo = (j % 2) * 64
                for c2 in range(2):
                    cs = slice(c2 * 512, (c2 + 1) * 512)
                    pp = ppp.tile([ob, 512], F32, name=nm("pp"), tag="pp")
                    for i in range(5):
                        nc.tensor.matmul(pp, wg[i][jo:jo + 64, :],
                                         hallT[i][p][jo:jo + 64, cs],
                                         start=(i == 0), stop=False)
                    nc.tensor.matmul(pp, wa[:, j * ob:(j + 1) * ob],
                                     stgall[:, cs], start=False, stop=True)
                    if gc == 0:
                        nc.scalar.activation(out=value[j][:, cs], in_=pp,
                                             func=AF.Sigmoid, bias=bias, scale=1.0)
                    else:
                        nc.scalar.activation(out=value[j][0:O2, cs], in_=pp,
                                             func=AF.Tanh, bias=bias, scale=1.0)

    # ================= gconv 1 =================
    if STAGE < 1:
        return
    t_slot(0, hxr)
    step(hA, hxr, s0t, None)
    t_slot(1, hA)
    step(hB, hA, s0t, hxr)
    t_slot(2, hB)
    step(hA, hxr, s1t, None)      # x3 reuses hA (x1 dead after t_slot(1))
    t_slot(3, hA)
    step(hB, hA, s1t, hxr)        # x4 reuses hB
    t_slot(4, hB)
    if STAGE < 2:
        return
    if rep == 0:
        a_family()
    if STAGE < 3:
        return
    project(0)
    if STAGE < 4:
        return

    # x'0 = r^T-transpose * hx (fused from PSUM); u -> node domain
    t_val(slice(0, 64), h0b, mul_by=hxr)
    t_val(slice(64, 128), un)
    if STAGE < 5:
        return

    # ================= gconv 2 =================
    t_slot(0, h0b)
    step(hA, h0b, s0t, None)
    t_slot(1, hA)
    step(hB, hA, s0t, h0b)
    t_slot(2, hB)
    step(hA, h0b, s1t, None)
    t_slot(3, hA)
    step(hB, hA, s1t, h0b)
    t_slot(4, hB)
    if STAGE < 6:
        return
    project(1)
    if STAGE < 7:
        return
    t_val(slice(0, 64), cn)
    if STAGE < 8:
        return

    # ---- final blend in node domain: out = c + u*(hx - c) ----
    for t in range(NT):
        nc.vector.tensor_sub(orow[t], hxrf[t], cn[t])
        nc.vector.tensor_mul(orow[t], un[t], orow[t])
        nc.vector.tensor_add(orow[t], cn[t], orow[t])
        if STAGE < 9:
            continue
        nc.sync.dma_start(
            out_d.rearrange("j (n u) -> n j u", u=U)[t * 128:(t + 1) * 128],
            orow[t].rearrange("p (j u) -> p j u", j=J),
        )


def _prep_shared(weights_output, biases_output, weights_update, biases_update):
    bf = ml_dtypes.bfloat16
    maps = {}
    for tag, W, ob in (("o", weights_output, O1), ("u", weights_update, O2)):
        Wr = W.reshape(66, 5, ob)
        H = Wr[2:, :, :]
        A = Wr[:2, :, :]
        for i in range(5):
            blk = np.concatenate([H[:, i], H[:, i]])   # rows duplicated at 0/64
            maps[f"w{tag}_g{i}"] = np.ascontiguousarray(blk).astype(bf)
        waj = np.zeros((80, J * ob), np.float32)
        for j in range(J):
            for m in range(5):
                for f in range(2):
                    waj[m * 16 + 2 * j + f, j * ob:(j + 1) * ob] = A[f, m]
        maps["waoj" if tag == "o" else "wauj"] = waj.astype(bf)
    maps["b_o"] = np.ascontiguousarray(biases_output.astype(np.float32)[:, None])
    maps["b_u"] = np.ascontiguousarray(biases_update.astype(np.float32)[:, None])
    maps["ident"] = np.eye(128, dtype=np.float32).astype(bf)
    maps["identb"] = np.concatenate([np.eye(64), np.eye(64)]).astype(bf)
    psel = np.zeros((16, 400), np.float32)
    for m in range(5):
        for i in range(16):
            psel[i, m * 80 + m * 16 + i] = 1.0
    maps["psel"] = psel.astype(bf)
    return maps


def make_in_maps(inputs, hx, support0, support1, weights_output, biases_output,
                 weights_update, biases_update):
    bf = ml_dtypes.bfloat16
    shared = _prep_shared(np.asarray(weights_output, dtype=np.float32),
                          np.asarray(biases_output, dtype=np.float32),
                          np.asarray(weights_update, dtype=np.float32),
                          np.asarray(biases_update, dtype=np.float32))
    shared["s0t"] = np.ascontiguousarray(np.asarray(support0, np.float32).T).astype(bf)
    shared["s1t"] = np.ascontiguousarray(np.asarray(support1, np.float32).T).astype(bf)

    hx = np.asarray(hx, dtype=np.float32)
    xi = np.asarray(inputs, dtype=np.float32).reshape(B, N, D_IN)
    hx3 = hx.reshape(B, N, U)

    in_maps = []
    for c in range(NCORES):
        sl = slice(c * J, (c + 1) * J)
        hxc = hx3[sl].transpose(1, 0, 2).reshape(N, J * U)
        a0 = xi[sl].transpose(1, 0, 2).reshape(N, 16)   # [n, (j,f)]
        m = dict(shared)
        m["hxr"] = hxc.astype(bf)
        m["hxrf"] = np.ascontiguousarray(hxc)
        m["a0r"] = a0.astype(bf)
        m["a0t"] = np.ascontiguousarray(a0.T).astype(bf)
        in_maps.append(m)
    return in_maps


def kernel(inputs, hx, support0, support1, weights_output, biases_output,
           weights_update, biases_update):
    if "nc" not in _CACHE:
        _CACHE["nc"] = _build()
    nc = _CACHE["nc"]
    in_maps = make_in_maps(inputs, hx, support0, support1, weights_output,
                           biases_output, weights_update, biases_update)
    res = run_bass_kernel_spmd(nc, in_maps, core_ids=list(range(NCORES)))
    return np.concatenate([r["out"] for r in res.results], axis=0)


# revision 14
# speedup vs baseline: 437.1800x; 437.1800x over previous
"""DCGRU cell on 8 Trainium2 NeuronCores — data-parallel over batch.

Problem: nn_DCGRUCell (B=64, N=1024, D_IN=2, U=64, K=2, 2 supports).
Sharding: batch 64 -> 8 cores x 8 local batches (j). Supports + weights
replicated per core; everything else fully local, no collectives.

v4: transposed-side diffusion. The Chebyshev recursion is folded into
precomputed matrices: with Q_i = (2*S_i^2 - I)^T (built on-device in the
rep-invariant preamble from S_i^T), every diffusion slot is a single
matmul family directly from x0:
    X_1^T = x0^T S0^T   X_2^T = x0^T Q_0   X_3^T = x0^T S1^T   X_4^T = x0^T Q_1
computed as lhsT = x0 (node-domain) against rhs = S^T/Q tiles, writing the
feature-domain hallT slots straight from PSUM — no per-slot transposes, no
node-domain slot state. Everything stays in SBUF; the only per-rep DMA is
the 8-tile output store.

Per-core layout:
  hxr[t]    [128, 512] bf16  node-domain hx (x0); cols j*64+u  (const)
  hxT[p]    [128,1024] bf16  feature-domain hx^T (const, preamble)
  h0b[t]    [128, 512] bf16  gconv2 x'0 = r*hx node-domain (lhsT)
  hallT[s][p] [128,1024] bf16 feature slots; rows (j%2)*64+u, j in {2p,2p+1}
            slot 0 holds x'0^T during gconv2 (gconv1 uses hxT)
  value[j]  [128, 1024] bf16 gconv1: rows 0:64=r^T, 64:128=u^T; gconv2
            tanh overwrites rows 0:64 with c^T
  s0q/s1q   [1024,1024] bf16 Q_i tiles (preamble: S node scratch in value)
Projection per (j, c2): 5 H-matmuls (k=64) + 1 A-matmul (k=80, per-j
weights). u/c return to node domain via 64-row PE transposes; blend in
node domain (bf16 hx) -> f32 orow -> out DMA.
"""

import numpy as np
import ml_dtypes

import concourse.bass as bass
import concourse.tile as tile
import concourse.mybir as mybir
from concourse import bacc
from concourse.bass_utils import run_bass_kernel_spmd

BF = mybir.dt.bfloat16
F32 = mybir.dt.float32
AF = mybir.ActivationFunctionType
OP = mybir.AluOpType

B, N, D_IN, U = 64, 1024, 2, 64
NCORES, J = 8, 8
NT = 8
O1, O2 = 2 * U, U

_CACHE = {}
STAGE = 99  # build cutoff for profiling components


def _build(reps=1, loop=False):
    """Build the kernel NEFF.

    reps>1 with loop=False unrolls the body (NEFF grows with reps).
    loop=True emits the rep-invariant preamble once, then the body inside
    a hardware For_i loop with trip count `reps` — constant NEFF size, so
    wall-clock slope over reps measures pure marginal execution cost.
    """
    nc = bacc.Bacc(None)

    s0t_d = nc.dram_tensor("s0t", [N, N], BF, kind="ExternalInput")
    s1t_d = nc.dram_tensor("s1t", [N, N], BF, kind="ExternalInput")
    hxr_d = nc.dram_tensor("hxr", [N, J * U], BF, kind="ExternalInput")
    a0r_d = nc.dram_tensor("a0r", [N, 16], BF, kind="ExternalInput")
    a0t_d = nc.dram_tensor("a0t", [16, N], BF, kind="ExternalInput")
    wo_g_d = [nc.dram_tensor(f"wo_g{g}", [128, O1], BF, kind="ExternalInput") for g in range(5)]
    wu_g_d = [nc.dram_tensor(f"wu_g{g}", [128, O2], BF, kind="ExternalInput") for g in range(5)]
    waoj_d = nc.dram_tensor("waoj", [80, J * O1], BF, kind="ExternalInput")
    wauj_d = nc.dram_tensor("wauj", [80, J * O2], BF, kind="ExternalInput")
    b_o_d = nc.dram_tensor("b_o", [O1, 1], F32, kind="ExternalInput")
    b_u_d = nc.dram_tensor("b_u", [O2, 1], F32, kind="ExternalInput")
    ident_d = nc.dram_tensor("ident", [128, 128], BF, kind="ExternalInput")
    identb_d = nc.dram_tensor("identb", [128, 64], BF, kind="ExternalInput")
    psel_d = nc.dram_tensor("psel", [16, 400], BF, kind="ExternalInput")
    out_d = nc.dram_tensor("out", [J, N * U], F32, kind="ExternalOutput")

    with tile.TileContext(nc) as tc:
        with (
            tc.tile_pool(name="const", bufs=1) as cp,
            tc.tile_pool(name="state", bufs=1) as hp,
            tc.tile_pool(name="orowp", bufs=4) as op_,
            tc.tile_pool(name="pd", bufs=3, space="PSUM") as pdp,
            tc.tile_pool(name="pt", bufs=3, space="PSUM") as ptp,
            tc.tile_pool(name="pp", bufs=2, space="PSUM") as ppp,
        ):
            env = {}
            # ---- constants ----
            s0t = [cp.tile([128, N], BF, name=f"s0t{k}") for k in range(NT)]
            s1t = [cp.tile([128, N], BF, name=f"s1t{k}") for k in range(NT)]
            for k in range(NT):
                nc.sync.dma_start(s0t[k], s0t_d[k * 128:(k + 1) * 128, :])
                nc.sync.dma_start(s1t[k], s1t_d[k * 128:(k + 1) * 128, :])
            s0q = [cp.tile([128, N], BF, name=f"s0q{k}") for k in range(NT)]
            s1q = [cp.tile([128, N], BF, name=f"s1q{k}") for k in range(NT)]
            wo_g = [cp.tile([128, O1], BF, name=f"wo_g{g}") for g in range(5)]
            wu_g = [cp.tile([128, O2], BF, name=f"wu_g{g}") for g in range(5)]
            for g in range(5):
                nc.sync.dma_start(wo_g[g], wo_g_d[g][:, :])
                nc.sync.dma_start(wu_g[g], wu_g_d[g][:, :])
            waoj = cp.tile([80, J * O1], BF, name="waoj")
            wauj = cp.tile([80, J * O2], BF, name="wauj")
            nc.sync.dma_start(waoj, waoj_d[:, :])
            nc.sync.dma_start(wauj, wauj_d[:, :])
            b_o = cp.tile([O1, 1], F32, name="b_o")
            b_u = cp.tile([O2, 1], F32, name="b_u")
            nc.sync.dma_start(b_o, b_o_d[:, :])
            nc.sync.dma_start(b_u, b_u_d[:, :])
            ident = cp.tile([128, 128], BF, name="ident")
            identb = cp.tile([128, 64], BF, name="identb")
            psel = cp.tile([16, 400], BF, name="psel")
            nc.sync.dma_start(ident, ident_d[:, :])
            nc.sync.dma_start(identb, identb_d[:, :])
            nc.sync.dma_start(psel, psel_d[:, :])

            hxr = [cp.tile([128, J * U], BF, name=f"hxr{t}") for t in range(NT)]
            hxT = [cp.tile([128, N], BF, name=f"hxT{p}") for p in range(4)]
            a0r = [cp.tile([128, 16], BF, name=f"a0r{t}") for t in range(NT)]
            a0t = cp.tile([16, N], BF, name="a0t")
            nc.sync.dma_start(a0t, a0t_d[:, :])
            for t in range(NT):
                nc.sync.dma_start(hxr[t], hxr_d[t * 128:(t + 1) * 128, :])
                nc.sync.dma_start(a0r[t], a0r_d[t * 128:(t + 1) * 128, :])

            # ---- persistent working state ----
            h0b = [hp.tile([128, J * U], BF, name=f"h0b{t}") for t in range(NT)]
            hallT = [[hp.tile([128, N], BF, name=f"hT{s}_{p}") for p in range(4)]
                     for s in range(5)]
            value = [hp.tile([128, N], BF, name=f"value{j}") for j in range(J)]
            un = [hp.tile([128, J * U], BF, name=f"un{t}") for t in range(NT)]
            cn = [hp.tile([128, J * U], BF, name=f"cn{t}") for t in range(NT)]
            # A-family (rep-invariant)
            amt = [a0t] + [cp.tile([16, N], BF, name=f"a{m}t") for m in range(1, 5)]
            a1r = [cp.tile([128, 16], BF, name=f"a1r{t}") for t in range(NT)]
            a3r = [cp.tile([128, 16], BF, name=f"a3r{t}") for t in range(NT)]
            stgall = cp.tile([80, N], BF, name="stgall")

            env.update(locals())
            if loop:
                _emit_pre(env)
                with tc.For_i(0, reps, 1):
                    _emit_body(env, 0, skip_pre=True)
            else:
                for rep in range(reps):
                    _emit_body(env, rep)
    nc.compile()
    return nc


def _emit_pre(env):
    """Rep-invariant preamble: A-family, hx^T, and Q = (2S^2-I)^T tiles."""
    nc = env["nc"]
    s0t, s1t = env["s0t"], env["s1t"]
    s0q, s1q = env["s0q"], env["s1q"]
    ident, identb, psel = env["ident"], env["identb"], env["psel"]
    hxr, hxT = env["hxr"], env["hxT"]
    a0r, a0t = env["a0r"], env["a0t"]
    amt, a1r, a3r, stgall = env["amt"], env["a1r"], env["a3r"], env["stgall"]
    value = env["value"]
    pdp, ptp, ppp = env["pdp"], env["ptp"], env["ppp"]
    uid = [0]

    def nm(pfx):
        uid[0] += 1
        return f"{pfx}P_{uid[0]}"

    # ---- A-family ^T tiles [16, 1024] + packed stgall ----
    for st, rowt, i1, i2 in ((s0t, a1r, 1, 2), (s1t, a3r, 3, 4)):
        for c2 in range(2):
            cs = slice(c2 * 512, (c2 + 1) * 512)
            pa = ppp.tile([16, 512], F32, name=nm("pa"), tag="pp")
            for kt in range(NT):
                nc.tensor.matmul(pa, a0r[kt], st[kt][:, cs],
                                 start=(kt == 0), stop=(kt == NT - 1))
            nc.any.tensor_copy(amt[i1][:, cs], pa)
        for t in range(NT):
            pr = ptp.tile([128, 16], BF, name=nm("pr"), tag="pt")
            nc.tensor.transpose(pr, amt[i1][:, t * 128:(t + 1) * 128],
                                identb[0:16, 0:16])
            nc.any.tensor_copy(rowt[t], pr)
        for c2 in range(2):
            cs = slice(c2 * 512, (c2 + 1) * 512)
            pa = ppp.tile([16, 512], F32, name=nm("pa2"), tag="pp")
            for kt in range(NT):
                nc.tensor.matmul(pa, rowt[kt], st[kt][:, cs],
                                 start=(kt == 0), stop=(kt == NT - 1))
            nc.vector.scalar_tensor_tensor(
                out=amt[i2][:, cs], in0=pa, scalar=2.0, in1=a0t[:, cs],
                op0=OP.mult, op1=OP.subtract)
    for c2 in range(2):
        cs = slice(c2 * 512, (c2 + 1) * 512)
        pk = ppp.tile([80, 512], F32, name=nm("pk"), tag="pp")
        for m in range(5):
            nc.tensor.matmul(pk, psel[:, m * 80:(m + 1) * 80], amt[m][:, cs],
                             start=(m == 0), stop=(m == 4))
        nc.any.tensor_copy(stgall[:, cs], pk)

    # ---- hx^T ----
    for p in range(4):
        for half in range(2):
            pt = ptp.tile([128, 512], BF, name=nm("px"), tag="pt")
            for ti in range(4):
                t = half * 4 + ti
                nc.tensor.transpose(pt[:, ti * 128:(ti + 1) * 128],
                                    hxr[t][:, p * 128:(p + 1) * 128], ident)
            nc.vector.tensor_copy(hxT[p][:, half * 512:(half + 1) * 512], pt)

    # ---- Q_i = 2*(S_i^T S_i^T) - I, using value tiles as S-node scratch ----
    for st, sq in ((s0t, s0q), (s1t, s1q)):
        for t in range(NT):
            for half in range(2):
                pt = ptp.tile([128, 512], BF, name=nm("sn"), tag="pt")
                for ki in range(4):
                    kt = half * 4 + ki
                    nc.tensor.transpose(pt[:, ki * 128:(ki + 1) * 128],
                                        st[kt][:, t * 128:(t + 1) * 128], ident)
                nc.vector.tensor_copy(
                    value[t][:, half * 512:(half + 1) * 512], pt)
        for a in range(NT):
            for half in range(2):
                cs = slice(half * 512, (half + 1) * 512)
                pq = pdp.tile([128, 512], F32, name=nm("pq"), tag="pd")
                for mt in range(NT):
                    nc.tensor.matmul(pq, value[mt][:, a * 128:(a + 1) * 128],
                                     st[mt][:, cs],
                                     start=(mt == 0), stop=(mt == NT - 1))
                nc.vector.tensor_scalar_mul(sq[a][:, cs], pq, 2.0)
            dcs = slice(a * 128, (a + 1) * 128)
            nc.vector.tensor_sub(sq[a][:, dcs], sq[a][:, dcs], ident)


def _emit_body(env, rep, skip_pre=False):
    nc = env["nc"]
    s0t, s1t = env["s0t"], env["s1t"]
    s0q, s1q = env["s0q"], env["s1q"]
    wo_g, wu_g = env["wo_g"], env["wu_g"]
    waoj, wauj = env["waoj"], env["wauj"]
    b_o, b_u = env["b_o"], env["b_u"]
    ident, identb = env["ident"], env["identb"]
    hxr, hxT = env["hxr"], env["hxT"]
    h0b = env["h0b"]
    hallT, value = env["hallT"], env["value"]
    un, cn = env["un"], env["cn"]
    op_ = env["op_"]
    pdp, ptp, ppp = env["pdp"], env["ptp"], env["ppp"]
    out_d = env["out_d"]
    R = f"r{rep}"
    uid = [0]

    def nm(pfx):
        uid[0] += 1
        return f"{pfx}{R}_{uid[0]}"

    def apply(s, x, M):
        # hallT[s] = x^T @ M  (lhsT = x node-domain, rhs = M tiles)
        for p in range(4):
            for half in range(2):
                cs = slice(half * 512, (half + 1) * 512)
                pd = pdp.tile([128, 512], F32, name=nm("pd"), tag="pd")
                for kt in range(NT):
                    nc.tensor.matmul(pd, x[kt][:, p * 128:(p + 1) * 128],
                                     M[kt][:, cs],
                                     start=(kt == 0), stop=(kt == NT - 1))
                nc.vector.tensor_copy(hallT[s][p][:, cs], pd)

    def t_slot(s, src):
        # hallT[s] = src^T  (node [1024, 512] -> feature [512, 1024])
        for p in range(4):
            for half in range(2):
                pt = ptp.tile([128, 512], BF, name=nm("pt"), tag="pt")
                for ti in range(4):
                    t = half * 4 + ti
                    nc.tensor.transpose(pt[:, ti * 128:(ti + 1) * 128],
                                        src[t][:, p * 128:(p + 1) * 128], ident)
                nc.vector.tensor_copy(
                    hallT[s][p][:, half * 512:(half + 1) * 512], pt)

    def t_val(rows, dst, mul_by=None):
        # dst[t][:, j*64:(j+1)*64] = value[j][rows].T  (64-row transposes);
        # with mul_by, dst[t] = transpose * mul_by[t] fused from PSUM.
        lo = rows.start
        for t in range(NT):
            pt = ptp.tile([128, 512], BF, name=nm("pv"), tag="pt")
            for j in range(J):
                nc.tensor.transpose(pt[:, j * 64:(j + 1) * 64],
                                    value[j][rows, t * 128:(t + 1) * 128],
                                    identb[lo:lo + 64, :])
            if mul_by is None:
                nc.vector.tensor_copy(dst[t], pt)
            else:
                nc.vector.tensor_mul(dst[t], pt, mul_by[t])

    def project(gc):
        wg, wa, ob, bias = ((wo_g, waoj, O1, b_o) if gc == 0
                            else (wu_g, wauj, O2, b_u))
        stgall = env["stgall"]
        for p in range(4):
            for j in (2 * p, 2 * p + 1):
                jo = (j % 2) * 64
                for c2 in range(2):
                    cs = slice(c2 * 512, (c2 + 1) * 512)
                    pp = ppp.tile([ob, 512], F32, name=nm("pp"), tag="pp")
                    for i in range(5):
                        rhs = (hxT[p] if (gc == 0 and i == 0)
                               else hallT[i][p] if i > 0 else hallT[0][p])
                        nc.tensor.matmul(pp, wg[i][jo:jo + 64, :],
                                         rhs[jo:jo + 64, cs],
                                         start=(i == 0), stop=False)
                    nc.tensor.matmul(pp, wa[:, j * ob:(j + 1) * ob],
                                     stgall[:, cs], start=False, stop=True)
                    if gc == 0:
                        nc.scalar.activation(out=value[j][:, cs], in_=pp,
                                             func=AF.Sigmoid, bias=bias, scale=1.0)
                    else:
                        nc.scalar.activation(out=value[j][0:O2, cs], in_=pp,
                                             func=AF.Tanh, bias=bias, scale=1.0)

    # ================= gconv 1 =================
    if STAGE < 1:
        return
    if rep == 0 and not skip_pre:
        _emit_pre(env)
    apply(1, hxr, s0t)
    apply(2, hxr, s0q)
    apply(3, hxr, s1t)
    apply(4, hxr, s1q)
    if STAGE < 3:
        return
    project(0)
    if STAGE < 4:
        return

    # x'0 = r^T-transpose * hx (fused from PSUM); u -> node domain
    t_val(slice(0, 64), h0b, mul_by=hxr)
    t_val(slice(64, 128), un)
    if STAGE < 5:
        return

    # ================= gconv 2 =================
    t_slot(0, h0b)
    apply(1, h0b, s0t)
    apply(2, h0b, s0q)
    apply(3, h0b, s1t)
    apply(4, h0b, s1q)
    if STAGE < 6:
        return
    project(1)
    if STAGE < 7:
        return
    t_val(slice(0, 64), cn)
    if STAGE < 8:
        return

    # ---- final blend in node domain: out = c + u*(hx - c) ----
    for t in range(NT):
        orow = op_.tile([128, J * U], F32, name=nm("or"), tag="orow")
        nc.vector.tensor_sub(orow, hxr[t], cn[t])
        nc.vector.tensor_mul(orow, un[t], orow)
        nc.vector.tensor_add(orow, cn[t], orow)
        if STAGE < 9:
            continue
        nc.sync.dma_start(
            out_d.rearrange("j (n u) -> n j u", u=U)[t * 128:(t + 1) * 128],
            orow.rearrange("p (j u) -> p j u", j=J),
        )


def _prep_shared(weights_output, biases_output, weights_update, biases_update):
    bf = ml_dtypes.bfloat16
    maps = {}
    for tag, W, ob in (("o", weights_output, O1), ("u", weights_update, O2)):
        Wr = W.reshape(66, 5, ob)
        H = Wr[2:, :, :]
        A = Wr[:2, :, :]
        for i in range(5):
            blk = np.concatenate([H[:, i], H[:, i]])   # rows duplicated at 0/64
            maps[f"w{tag}_g{i}"] = np.ascontiguousarray(blk).astype(bf)
        waj = np.zeros((80, J * ob), np.float32)
        for j in range(J):
            for m in range(5):
                for f in range(2):
                    waj[m * 16 + 2 * j + f, j * ob:(j + 1) * ob] = A[f, m]
        maps["waoj" if tag == "o" else "wauj"] = waj.astype(bf)
    maps["b_o"] = np.ascontiguousarray(biases_output.astype(np.float32)[:, None])
    maps["b_u"] = np.ascontiguousarray(biases_update.astype(np.float32)[:, None])
    maps["ident"] = np.eye(128, dtype=np.float32).astype(bf)
    maps["identb"] = np.concatenate([np.eye(64), np.eye(64)]).astype(bf)
    psel = np.zeros((16, 400), np.float32)
    for m in range(5):
        for i in range(16):
            psel[i, m * 80 + m * 16 + i] = 1.0
    maps["psel"] = psel.astype(bf)
    return maps


def make_in_maps(inputs, hx, support0, support1, weights_output, biases_output,
                 weights_update, biases_update):
    bf = ml_dtypes.bfloat16
    shared = _prep_shared(np.asarray(weights_output, dtype=np.float32),
                          np.asarray(biases_output, dtype=np.float32),
                          np.asarray(weights_update, dtype=np.float32),
                          np.asarray(biases_update, dtype=np.float32))
    shared["s0t"] = np.ascontiguousarray(np.asarray(support0, np.float32).T).astype(bf)
    shared["s1t"] = np.ascontiguousarray(np.asarray(support1, np.float32).T).astype(bf)

    hx = np.asarray(hx, dtype=np.float32)
    xi = np.asarray(inputs, dtype=np.float32).reshape(B, N, D_IN)
    hx3 = hx.reshape(B, N, U)

    in_maps = []
    for c in range(NCORES):
        sl = slice(c * J, (c + 1) * J)
        hxc = hx3[sl].transpose(1, 0, 2).reshape(N, J * U)
        a0 = xi[sl].transpose(1, 0, 2).reshape(N, 16)   # [n, (j,f)]
        m = dict(shared)
        m["hxr"] = hxc.astype(bf)
        m["a0r"] = a0.astype(bf)
        m["a0t"] = np.ascontiguousarray(a0.T).astype(bf)
        in_maps.append(m)
    return in_maps


def kernel(inputs, hx, support0, support1, weights_output, biases_output,
           weights_update, biases_update):
    if "nc" not in _CACHE:
        _CACHE["nc"] = _build()
    nc = _CACHE["nc"]
    in_maps = make_in_maps(inputs, hx, support0, support1, weights_output,
                           biases_output, weights_update, biases_update)
    res = run_bass_kernel_spmd(nc, in_maps, core_ids=list(range(NCORES)))
    return np.concatenate([r["out"] for r in res.results], axis=0)


# revision 16
# speedup vs baseline: 440.9401x; 1.0086x over previous
"""DCGRU cell on 8 Trainium2 NeuronCores — data-parallel over batch.

Problem: nn_DCGRUCell (B=64, N=1024, D_IN=2, U=64, K=2, 2 supports).
Sharding: batch 64 -> 8 cores x 8 local batches (j). Supports + weights
replicated per core; everything else fully local, no collectives.

v4: transposed-side diffusion. The Chebyshev recursion is folded into
precomputed matrices: with Q_i = (2*S_i^2 - I)^T (built on-device in the
rep-invariant preamble from S_i^T), every diffusion slot is a single
matmul family directly from x0:
    X_1^T = x0^T S0^T   X_2^T = x0^T Q_0   X_3^T = x0^T S1^T   X_4^T = x0^T Q_1
computed as lhsT = x0 (node-domain) against rhs = S^T/Q tiles, writing the
feature-domain hallT slots straight from PSUM — no per-slot transposes, no
node-domain slot state. Everything stays in SBUF; the only per-rep DMA is
the 8-tile output store.

Per-core layout:
  hxr[t]    [128, 512] bf16  node-domain hx (x0); cols j*64+u  (const)
  hxT[p]    [128,1024] bf16  feature-domain hx^T (const, preamble)
  h0b[t]    [128, 512] bf16  gconv2 x'0 = r*hx node-domain (lhsT)
  hallT[s][p] [128,1024] bf16 feature slots; rows (j%2)*64+u, j in {2p,2p+1}
            slot 0 holds x'0^T during gconv2 (gconv1 uses hxT)
  value[j]  [128, 1024] bf16 gconv1: rows 0:64=r^T, 64:128=u^T; gconv2
            tanh overwrites rows 0:64 with c^T
  s0q/s1q   [1024,1024] bf16 Q_i tiles (preamble: S node scratch in value)
Projection per (j, c2): 5 H-matmuls (k=64) + 1 A-matmul (k=80, per-j
weights). u/c return to node domain via 64-row PE transposes; blend in
node domain (bf16 hx) -> f32 orow -> out DMA.
"""

import numpy as np
import ml_dtypes

import concourse.bass as bass
import concourse.tile as tile
import concourse.mybir as mybir
from concourse import bacc
from concourse.bass_utils import run_bass_kernel_spmd

BF = mybir.dt.bfloat16
F32 = mybir.dt.float32
AF = mybir.ActivationFunctionType
OP = mybir.AluOpType

B, N, D_IN, U = 64, 1024, 2, 64
NCORES, J = 8, 8
NT = 8
O1, O2 = 2 * U, U

_CACHE = {}
STAGE = 99  # build cutoff for profiling components


def _build(reps=1, loop=False):
    """Build the kernel NEFF.

    reps>1 with loop=False unrolls the body (NEFF grows with reps).
    loop=True emits the rep-invariant preamble once, then the body inside
    a hardware For_i loop with trip count `reps` — constant NEFF size, so
    wall-clock slope over reps measures pure marginal execution cost.
    """
    nc = bacc.Bacc(None)

    s0t_d = nc.dram_tensor("s0t", [N, N], BF, kind="ExternalInput")
    s1t_d = nc.dram_tensor("s1t", [N, N], BF, kind="ExternalInput")
    hxr_d = nc.dram_tensor("hxr", [N, J * U], BF, kind="ExternalInput")
    a0r_d = nc.dram_tensor("a0r", [N, 16], BF, kind="ExternalInput")
    a0t_d = nc.dram_tensor("a0t", [16, N], BF, kind="ExternalInput")
    wo_g_d = [nc.dram_tensor(f"wo_g{g}", [128, O1], BF, kind="ExternalInput") for g in range(5)]
    wu_g_d = [nc.dram_tensor(f"wu_g{g}", [128, O2], BF, kind="ExternalInput") for g in range(5)]
    waoj_d = nc.dram_tensor("waoj", [80, J * O1], BF, kind="ExternalInput")
    wauj_d = nc.dram_tensor("wauj", [80, J * O2], BF, kind="ExternalInput")
    b_o_d = nc.dram_tensor("b_o", [O1, 1], F32, kind="ExternalInput")
    b_u_d = nc.dram_tensor("b_u", [O2, 1], F32, kind="ExternalInput")
    ident_d = nc.dram_tensor("ident", [128, 128], BF, kind="ExternalInput")
    identb_d = nc.dram_tensor("identb", [128, 64], BF, kind="ExternalInput")
    psel_d = nc.dram_tensor("psel", [16, 400], BF, kind="ExternalInput")
    out_d = nc.dram_tensor("out", [J, N * U], F32, kind="ExternalOutput")

    with tile.TileContext(nc) as tc:
        with (
            tc.tile_pool(name="const", bufs=1) as cp,
            tc.tile_pool(name="state", bufs=1) as hp,
            tc.tile_pool(name="orowp", bufs=4) as op_,
            tc.tile_pool(name="pd", bufs=4, space="PSUM") as pdp,
            tc.tile_pool(name="pt", bufs=2, space="PSUM") as ptp,
            tc.tile_pool(name="pp", bufs=2, space="PSUM") as ppp,
        ):
            env = {}
            # ---- constants ----
            s0t = [cp.tile([128, N], BF, name=f"s0t{k}") for k in range(NT)]
            s1t = [cp.tile([128, N], BF, name=f"s1t{k}") for k in range(NT)]
            for k in range(NT):
                nc.sync.dma_start(s0t[k], s0t_d[k * 128:(k + 1) * 128, :])
                nc.sync.dma_start(s1t[k], s1t_d[k * 128:(k + 1) * 128, :])
            s0q = [cp.tile([128, N], BF, name=f"s0q{k}") for k in range(NT)]
            s1q = [cp.tile([128, N], BF, name=f"s1q{k}") for k in range(NT)]
            wo_g = [cp.tile([128, O1], BF, name=f"wo_g{g}") for g in range(5)]
            wu_g = [cp.tile([128, O2], BF, name=f"wu_g{g}") for g in range(5)]
            for g in range(5):
                nc.sync.dma_start(wo_g[g], wo_g_d[g][:, :])
                nc.sync.dma_start(wu_g[g], wu_g_d[g][:, :])
            waoj = cp.tile([80, J * O1], BF, name="waoj")
            wauj = cp.tile([80, J * O2], BF, name="wauj")
            nc.sync.dma_start(waoj, waoj_d[:, :])
            nc.sync.dma_start(wauj, wauj_d[:, :])
            b_o = cp.tile([O1, 1], F32, name="b_o")
            b_u = cp.tile([O2, 1], F32, name="b_u")
            nc.sync.dma_start(b_o, b_o_d[:, :])
            nc.sync.dma_start(b_u, b_u_d[:, :])
            ident = cp.tile([128, 128], BF, name="ident")
            identb = cp.tile([128, 64], BF, name="identb")
            psel = cp.tile([16, 400], BF, name="psel")
            nc.sync.dma_start(ident, ident_d[:, :])
            nc.sync.dma_start(identb, identb_d[:, :])
            nc.sync.dma_start(psel, psel_d[:, :])

            hxr = [cp.tile([128, J * U], BF, name=f"hxr{t}") for t in range(NT)]
            hxT = [cp.tile([128, N], BF, name=f"hxT{p}") for p in range(4)]
            a0r = [cp.tile([128, 16], BF, name=f"a0r{t}") for t in range(NT)]
            a0t = cp.tile([16, N], BF, name="a0t")
            nc.sync.dma_start(a0t, a0t_d[:, :])
            for t in range(NT):
                nc.sync.dma_start(hxr[t], hxr_d[t * 128:(t + 1) * 128, :])
                nc.sync.dma_start(a0r[t], a0r_d[t * 128:(t + 1) * 128, :])

            # ---- persistent working state ----
            h0b = [hp.tile([128, J * U], BF, name=f"h0b{t}") for t in range(NT)]
            hallT = [[hp.tile([128, N], BF, name=f"hT{s}_{p}") for p in range(4)]
                     for s in range(5)]
            value = [hp.tile([128, N], BF, name=f"value{j}") for j in range(J)]
            un = [hp.tile([128, J * U], BF, name=f"un{t}") for t in range(NT)]
            cn = [hp.tile([128, J * U], BF, name=f"cn{t}") for t in range(NT)]
            # A-family (rep-invariant)
            amt = [a0t] + [cp.tile([16, N], BF, name=f"a{m}t") for m in range(1, 5)]
            a1r = [cp.tile([128, 16], BF, name=f"a1r{t}") for t in range(NT)]
            a3r = [cp.tile([128, 16], BF, name=f"a3r{t}") for t in range(NT)]
            stgall = cp.tile([80, N], BF, name="stgall")

            env.update(locals())
            if loop:
                _emit_pre(env)
                with tc.For_i(0, reps, 1):
                    _emit_body(env, 0, skip_pre=True)
            else:
                for rep in range(reps):
                    _emit_body(env, rep)
    nc.compile()
    return nc


def _emit_pre(env):
    """Rep-invariant preamble: A-family, hx^T, and Q = (2S^2-I)^T tiles."""
    nc = env["nc"]
    s0t, s1t = env["s0t"], env["s1t"]
    s0q, s1q = env["s0q"], env["s1q"]
    ident, identb, psel = env["ident"], env["identb"], env["psel"]
    hxr, hxT = env["hxr"], env["hxT"]
    a0r, a0t = env["a0r"], env["a0t"]
    amt, a1r, a3r, stgall = env["amt"], env["a1r"], env["a3r"], env["stgall"]
    value = env["value"]
    pdp, ptp, ppp = env["pdp"], env["ptp"], env["ppp"]
    uid = [0]

    def nm(pfx):
        uid[0] += 1
        return f"{pfx}P_{uid[0]}"

    # ---- A-family ^T tiles [16, 1024] + packed stgall ----
    for st, rowt, i1, i2 in ((s0t, a1r, 1, 2), (s1t, a3r, 3, 4)):
        for c2 in range(2):
            cs = slice(c2 * 512, (c2 + 1) * 512)
            pa = ppp.tile([16, 512], F32, name=nm("pa"), tag="pp")
            for kt in range(NT):
                nc.tensor.matmul(pa, a0r[kt], st[kt][:, cs],
                                 start=(kt == 0), stop=(kt == NT - 1))
            nc.any.tensor_copy(amt[i1][:, cs], pa)
        for t in range(NT):
            pr = ptp.tile([128, 16], BF, name=nm("pr"), tag="pt")
            nc.tensor.transpose(pr, amt[i1][:, t * 128:(t + 1) * 128],
                                identb[0:16, 0:16])
            nc.any.tensor_copy(rowt[t], pr)
        for c2 in range(2):
            cs = slice(c2 * 512, (c2 + 1) * 512)
            pa = ppp.tile([16, 512], F32, name=nm("pa2"), tag="pp")
            for kt in range(NT):
                nc.tensor.matmul(pa, rowt[kt], st[kt][:, cs],
                                 start=(kt == 0), stop=(kt == NT - 1))
            nc.vector.scalar_tensor_tensor(
                out=amt[i2][:, cs], in0=pa, scalar=2.0, in1=a0t[:, cs],
                op0=OP.mult, op1=OP.subtract)
    for c2 in range(2):
        cs = slice(c2 * 512, (c2 + 1) * 512)
        pk = ppp.tile([80, 512], F32, name=nm("pk"), tag="pp")
        for m in range(5):
            nc.tensor.matmul(pk, psel[:, m * 80:(m + 1) * 80], amt[m][:, cs],
                             start=(m == 0), stop=(m == 4))
        nc.any.tensor_copy(stgall[:, cs], pk)

    # ---- hx^T ----
    for p in range(4):
        for half in range(2):
            pt = ptp.tile([128, 512], BF, name=nm("px"), tag="pt")
            for ti in range(4):
                t = half * 4 + ti
                nc.tensor.transpose(pt[:, ti * 128:(ti + 1) * 128],
                                    hxr[t][:, p * 128:(p + 1) * 128], ident)
            nc.vector.tensor_copy(hxT[p][:, half * 512:(half + 1) * 512], pt)

    # ---- Q_i = 2*(S_i^T S_i^T) - I, using value tiles as S-node scratch ----
    for st, sq in ((s0t, s0q), (s1t, s1q)):
        for t in range(NT):
            for half in range(2):
                pt = ptp.tile([128, 512], BF, name=nm("sn"), tag="pt")
                for ki in range(4):
                    kt = half * 4 + ki
                    nc.tensor.transpose(pt[:, ki * 128:(ki + 1) * 128],
                                        st[kt][:, t * 128:(t + 1) * 128], ident)
                nc.vector.tensor_copy(
                    value[t][:, half * 512:(half + 1) * 512], pt)
        for a in range(NT):
            for half in range(2):
                cs = slice(half * 512, (half + 1) * 512)
                pq = pdp.tile([128, 512], F32, name=nm("pq"), tag="pd")
                for mt in range(NT):
                    nc.tensor.matmul(pq, value[mt][:, a * 128:(a + 1) * 128],
                                     st[mt][:, cs],
                                     start=(mt == 0), stop=(mt == NT - 1))
                nc.vector.tensor_scalar_mul(sq[a][:, cs], pq, 2.0)
            dcs = slice(a * 128, (a + 1) * 128)
            nc.vector.tensor_sub(sq[a][:, dcs], sq[a][:, dcs], ident)


def _emit_body(env, rep, skip_pre=False):
    nc = env["nc"]
    s0t, s1t = env["s0t"], env["s1t"]
    s0q, s1q = env["s0q"], env["s1q"]
    wo_g, wu_g = env["wo_g"], env["wu_g"]
    waoj, wauj = env["waoj"], env["wauj"]
    b_o, b_u = env["b_o"], env["b_u"]
    ident, identb = env["ident"], env["identb"]
    hxr, hxT = env["hxr"], env["hxT"]
    h0b = env["h0b"]
    hallT, value = env["hallT"], env["value"]
    un, cn = env["un"], env["cn"]
    op_ = env["op_"]
    pdp, ptp, ppp = env["pdp"], env["ptp"], env["ppp"]
    out_d = env["out_d"]
    R = f"r{rep}"
    uid = [0]

    def nm(pfx):
        uid[0] += 1
        return f"{pfx}{R}_{uid[0]}"

    def apply(s, x, M):
        # hallT[s] = x^T @ M  (lhsT = x node-domain, rhs = M tiles).
        # kt-outer over the two 512-col halves: consecutive matmuls share
        # the same stationary operand x[kt] chunk.
        for p in range(4):
            pd = [pdp.tile([128, 512], F32, name=nm("pd"), tag="pd")
                  for _ in range(2)]
            for kt in range(NT):
                for half in range(2):
                    cs = slice(half * 512, (half + 1) * 512)
                    nc.tensor.matmul(pd[half], x[kt][:, p * 128:(p + 1) * 128],
                                     M[kt][:, cs],
                                     start=(kt == 0), stop=(kt == NT - 1))
            for half in range(2):
                cs = slice(half * 512, (half + 1) * 512)
                nc.vector.tensor_copy(hallT[s][p][:, cs], pd[half])

    def t_slot(s, src):
        # hallT[s] = src^T  (node [1024, 512] -> feature [512, 1024])
        for p in range(4):
            for half in range(2):
                pt = ptp.tile([128, 512], BF, name=nm("pt"), tag="pt")
                for ti in range(4):
                    t = half * 4 + ti
                    nc.tensor.transpose(pt[:, ti * 128:(ti + 1) * 128],
                                        src[t][:, p * 128:(p + 1) * 128], ident)
                nc.vector.tensor_copy(
                    hallT[s][p][:, half * 512:(half + 1) * 512], pt)

    def t_val(rows, dst, mul_by=None):
        # dst[t][:, j*64:(j+1)*64] = value[j][rows].T  (64-row transposes);
        # with mul_by, dst[t] = transpose * mul_by[t] fused from PSUM.
        lo = rows.start
        for t in range(NT):
            pt = ptp.tile([128, 512], BF, name=nm("pv"), tag="pt")
            for j in range(J):
                nc.tensor.transpose(pt[:, j * 64:(j + 1) * 64],
                                    value[j][rows, t * 128:(t + 1) * 128],
                                    identb[lo:lo + 64, :])
            if mul_by is None:
                nc.vector.tensor_copy(dst[t], pt)
            else:
                nc.vector.tensor_mul(dst[t], pt, mul_by[t])

    def project(gc):
        wg, wa, ob, bias = ((wo_g, waoj, O1, b_o) if gc == 0
                            else (wu_g, wauj, O2, b_u))
        stgall = env["stgall"]
        for p in range(4):
            for j in (2 * p, 2 * p + 1):
                jo = (j % 2) * 64
                for c2 in range(2):
                    cs = slice(c2 * 512, (c2 + 1) * 512)
                    pp = ppp.tile([ob, 512], F32, name=nm("pp"), tag="pp")
                    for i in range(5):
                        rhs = (hxT[p] if (gc == 0 and i == 0)
                               else hallT[i][p] if i > 0 else hallT[0][p])
                        nc.tensor.matmul(pp, wg[i][jo:jo + 64, :],
                                         rhs[jo:jo + 64, cs],
                                         start=(i == 0), stop=False)
                    nc.tensor.matmul(pp, wa[:, j * ob:(j + 1) * ob],
                                     stgall[:, cs], start=False, stop=True)
                    if gc == 0:
                        nc.scalar.activation(out=value[j][:, cs], in_=pp,
                                             func=AF.Sigmoid, bias=bias, scale=1.0)
                    else:
                        nc.scalar.activation(out=value[j][0:O2, cs], in_=pp,
                                             func=AF.Tanh, bias=bias, scale=1.0)

    # ================= gconv 1 =================
    if STAGE < 1:
        return
    if rep == 0 and not skip_pre:
        _emit_pre(env)
    apply(1, hxr, s0t)
    apply(2, hxr, s0q)
    apply(3, hxr, s1t)
    apply(4, hxr, s1q)
    if STAGE < 3:
        return
    project(0)
    if STAGE < 4:
        return

    # x'0 = r^T-transpose * hx (fused from PSUM); u -> node domain
    t_val(slice(0, 64), h0b, mul_by=hxr)
    t_val(slice(64, 128), un)
    if STAGE < 5:
        return

    # ================= gconv 2 =================
    t_slot(0, h0b)
    apply(1, h0b, s0t)
    apply(2, h0b, s0q)
    apply(3, h0b, s1t)
    apply(4, h0b, s1q)
    if STAGE < 6:
        return
    project(1)
    if STAGE < 7:
        return
    t_val(slice(0, 64), cn)
    if STAGE < 8:
        return

    # ---- final blend in node domain: out = c + u*(hx - c) ----
    for t in range(NT):
        orow = op_.tile([128, J * U], F32, name=nm("or"), tag="orow")
        nc.vector.tensor_sub(orow, hxr[t], cn[t])
        nc.vector.tensor_mul(orow, un[t], orow)
        nc.vector.tensor_add(orow, cn[t], orow)
        if STAGE < 9:
            continue
        nc.sync.dma_start(
            out_d.rearrange("j (n u) -> n j u", u=U)[t * 128:(t + 1) * 128],
            orow.rearrange("p (j u) -> p j u", j=J),
        )


def _prep_shared(weights_output, biases_output, weights_update, biases_update):
    bf = ml_dtypes.bfloat16
    maps = {}
    for tag, W, ob in (("o", weights_output, O1), ("u", weights_update, O2)):
        Wr = W.reshape(66, 5, ob)
        H = Wr[2:, :, :]
        A = Wr[:2, :, :]
        for i in range(5):
            blk = np.concatenate([H[:, i], H[:, i]])   # rows duplicated at 0/64
            maps[f"w{tag}_g{i}"] = np.ascontiguousarray(blk).astype(bf)
        waj = np.zeros((80, J * ob), np.float32)
        for j in range(J):
            for m in range(5):
                for f in range(2):
                    waj[m * 16 + 2 * j + f, j * ob:(j + 1) * ob] = A[f, m]
        maps["waoj" if tag == "o" else "wauj"] = waj.astype(bf)
    maps["b_o"] = np.ascontiguousarray(biases_output.astype(np.float32)[:, None])
    maps["b_u"] = np.ascontiguousarray(biases_update.astype(np.float32)[:, None])
    maps["ident"] = np.eye(128, dtype=np.float32).astype(bf)
    maps["identb"] = np.concatenate([np.eye(64), np.eye(64)]).astype(bf)
    psel = np.zeros((16, 400), np.float32)
    for m in range(5):
        for i in range(16):
            psel[i, m * 80 + m * 16 + i] = 1.0
    maps["psel"] = psel.astype(bf)
    return maps


def make_in_maps(inputs, hx, support0, support1, weights_output, biases_output,
                 weights_update, biases_update):
    bf = ml_dtypes.bfloat16
    shared = _prep_shared(np.asarray(weights_output, dtype=np.float32),
                          np.asarray(biases_output, dtype=np.float32),
                          np.asarray(weights_update, dtype=np.float32),
                          np.asarray(biases_update, dtype=np.float32))
    shared["s0t"] = np.ascontiguousarray(np.asarray(support0, np.float32).T).astype(bf)
    shared["s1t"] = np.ascontiguousarray(np.asarray(support1, np.float32).T).astype(bf)

    hx = np.asarray(hx, dtype=np.float32)
    xi = np.asarray(inputs, dtype=np.float32).reshape(B, N, D_IN)
    hx3 = hx.reshape(B, N, U)

    in_maps = []
    for c in range(NCORES):
        sl = slice(c * J, (c + 1) * J)
        hxc = hx3[sl].transpose(1, 0, 2).reshape(N, J * U)
        a0 = xi[sl].transpose(1, 0, 2).reshape(N, 16)   # [n, (j,f)]
        m = dict(shared)
        m["hxr"] = hxc.astype(bf)
        m["a0r"] = a0.astype(bf)
        m["a0t"] = np.ascontiguousarray(a0.T).astype(bf)
        in_maps.append(m)
    return in_maps


def kernel(inputs, hx, support0, support1, weights_output, biases_output,
           weights_update, biases_update):
    if "nc" not in _CACHE:
        _CACHE["nc"] = _build()
    nc = _CACHE["nc"]
    in_maps = make_in_maps(inputs, hx, support0, support1, weights_output,
                           biases_output, weights_update, biases_update)
    res = run_bass_kernel_spmd(nc, in_maps, core_ids=list(range(NCORES)))
    return np.concatenate([r["out"] for r in res.results], axis=0)


# revision 19
# speedup vs baseline: 538.9376x; 1.2222x over previous
"""DCGRU cell on 8 Trainium2 NeuronCores — data-parallel over batch.

Problem: nn_DCGRUCell (B=64, N=1024, D_IN=2, U=64, K=2, 2 supports).
Sharding: batch 64 -> 8 cores x 8 local batches (j). Supports + weights
replicated per core; everything else fully local, no collectives.

v4: transposed-side diffusion. The Chebyshev recursion is folded into
precomputed matrices: with Q_i = (2*S_i^2 - I)^T (built on-device in the
rep-invariant preamble from S_i^T), every diffusion slot is a single
matmul family directly from x0:
    X_1^T = x0^T S0^T   X_2^T = x0^T Q_0   X_3^T = x0^T S1^T   X_4^T = x0^T Q_1
computed as lhsT = x0 (node-domain) against rhs = S^T/Q tiles, writing the
feature-domain hallT slots straight from PSUM — no per-slot transposes, no
node-domain slot state. Everything stays in SBUF; the only per-rep DMA is
the 8-tile output store.

Per-core layout:
  hxr[t]    [128, 512] bf16  node-domain hx (x0); cols j*64+u  (const)
  hxT[p]    [128,1024] bf16  feature-domain hx^T (const, preamble)
  h0b[t]    [128, 512] bf16  gconv2 x'0 = r*hx node-domain (lhsT)
  hallT[s][p] [128,1024] bf16 feature slots; rows (j%2)*64+u, j in {2p,2p+1}
            slot 0 holds x'0^T during gconv2 (gconv1 uses hxT)
  value[j]  [128, 1024] bf16 gconv1: rows 0:64=r^T, 64:128=u^T; gconv2
            tanh overwrites rows 0:64 with c^T
  s0q/s1q   [1024,1024] bf16 Q_i tiles (preamble: S node scratch in value)
Projection per (j, c2): 5 H-matmuls (k=64) + 1 A-matmul (k=80, per-j
weights). u/c return to node domain via 64-row PE transposes; blend in
node domain (bf16 hx) -> f32 orow -> out DMA.
"""

import numpy as np
import ml_dtypes

import concourse.bass as bass
import concourse.tile as tile
import concourse.mybir as mybir
from concourse import bacc
from concourse.bass_utils import run_bass_kernel_spmd

BF = mybir.dt.bfloat16
F32 = mybir.dt.float32
AF = mybir.ActivationFunctionType
OP = mybir.AluOpType

B, N, D_IN, U = 64, 1024, 2, 64
NCORES, J = 8, 8
NT = 8
O1, O2 = 2 * U, U

_CACHE = {}
STAGE = 99  # build cutoff for profiling components


def _build(reps=1, loop=False):
    """Build the kernel NEFF.

    reps>1 with loop=False unrolls the body (NEFF grows with reps).
    loop=True emits the rep-invariant preamble once, then the body inside
    a hardware For_i loop with trip count `reps` — constant NEFF size, so
    wall-clock slope over reps measures pure marginal execution cost.
    """
    nc = bacc.Bacc(None)

    s0t_d = nc.dram_tensor("s0t", [N, N], BF, kind="ExternalInput")
    s1t_d = nc.dram_tensor("s1t", [N, N], BF, kind="ExternalInput")
    hxr_d = nc.dram_tensor("hxr", [N, J * U], BF, kind="ExternalInput")
    a0r_d = nc.dram_tensor("a0r", [N, 16], BF, kind="ExternalInput")
    a0t_d = nc.dram_tensor("a0t", [16, N], BF, kind="ExternalInput")
    wo_g_d = [nc.dram_tensor(f"wo_g{g}", [128, O1], BF, kind="ExternalInput") for g in range(5)]
    wu_g_d = [nc.dram_tensor(f"wu_g{g}", [128, O2], BF, kind="ExternalInput") for g in range(5)]
    waoj_d = nc.dram_tensor("waoj", [80, J * O1], BF, kind="ExternalInput")
    wauj_d = nc.dram_tensor("wauj", [80, J * O2], BF, kind="ExternalInput")
    b_o_d = nc.dram_tensor("b_o", [O1, 1], F32, kind="ExternalInput")
    b_u_d = nc.dram_tensor("b_u", [O2, 1], F32, kind="ExternalInput")
    ident_d = nc.dram_tensor("ident", [128, 128], BF, kind="ExternalInput")
    identb_d = nc.dram_tensor("identb", [128, 64], BF, kind="ExternalInput")
    psel_d = nc.dram_tensor("psel", [16, 400], BF, kind="ExternalInput")
    out_d = nc.dram_tensor("out", [J, N * U], F32, kind="ExternalOutput")

    with tile.TileContext(nc) as tc:
        with (
            tc.tile_pool(name="const", bufs=1) as cp,
            tc.tile_pool(name="state", bufs=1) as hp,
            tc.tile_pool(name="orowp", bufs=4) as op_,
            tc.tile_pool(name="pd", bufs=4, space="PSUM") as pdp,
            tc.tile_pool(name="pt", bufs=2, space="PSUM") as ptp,
            tc.tile_pool(name="pp", bufs=2, space="PSUM") as ppp,
        ):
            env = {}
            # ---- constants ----
            s0t = [cp.tile([128, N], BF, name=f"s0t{k}") for k in range(NT)]
            s1t = [cp.tile([128, N], BF, name=f"s1t{k}") for k in range(NT)]
            for k in range(NT):
                nc.sync.dma_start(s0t[k], s0t_d[k * 128:(k + 1) * 128, :])
                nc.sync.dma_start(s1t[k], s1t_d[k * 128:(k + 1) * 128, :])
            s0q = [cp.tile([128, N], BF, name=f"s0q{k}") for k in range(NT)]
            s1q = [cp.tile([128, N], BF, name=f"s1q{k}") for k in range(NT)]
            wo_g = [cp.tile([128, O1], BF, name=f"wo_g{g}") for g in range(5)]
            wu_g = [cp.tile([128, O2], BF, name=f"wu_g{g}") for g in range(5)]
            for g in range(5):
                nc.sync.dma_start(wo_g[g], wo_g_d[g][:, :])
                nc.sync.dma_start(wu_g[g], wu_g_d[g][:, :])
            waoj = cp.tile([80, J * O1], BF, name="waoj")
            wauj = cp.tile([80, J * O2], BF, name="wauj")
            nc.sync.dma_start(waoj, waoj_d[:, :])
            nc.sync.dma_start(wauj, wauj_d[:, :])
            b_o = cp.tile([O1, 1], F32, name="b_o")
            b_u = cp.tile([O2, 1], F32, name="b_u")
            nc.sync.dma_start(b_o, b_o_d[:, :])
            nc.sync.dma_start(b_u, b_u_d[:, :])
            ident = cp.tile([128, 128], BF, name="ident")
            identb = cp.tile([128, 64], BF, name="identb")
            psel = cp.tile([16, 400], BF, name="psel")
            nc.sync.dma_start(ident, ident_d[:, :])
            nc.sync.dma_start(identb, identb_d[:, :])
            nc.sync.dma_start(psel, psel_d[:, :])

            hxr = [cp.tile([128, J * U], BF, name=f"hxr{t}") for t in range(NT)]
            hxT = [cp.tile([128, N], BF, name=f"hxT{p}") for p in range(4)]
            a0r = [cp.tile([128, 16], BF, name=f"a0r{t}") for t in range(NT)]
            a0t = cp.tile([16, N], BF, name="a0t")
            nc.sync.dma_start(a0t, a0t_d[:, :])
            for t in range(NT):
                nc.sync.dma_start(hxr[t], hxr_d[t * 128:(t + 1) * 128, :])
                nc.sync.dma_start(a0r[t], a0r_d[t * 128:(t + 1) * 128, :])

            # ---- persistent working state ----
            h0b = [hp.tile([128, J * U], BF, name=f"h0b{t}") for t in range(NT)]
            hallT = [[hp.tile([128, N], BF, name=f"hT{s}_{p}") for p in range(4)]
                     for s in range(5)]
            value = [hp.tile([128, N], BF, name=f"value{j}") for j in range(J)]
            un = [hp.tile([128, J * U], BF, name=f"un{t}") for t in range(NT)]
            cn = [hp.tile([128, J * U], BF, name=f"cn{t}") for t in range(NT)]
            # A-family (rep-invariant)
            amt = [a0t] + [cp.tile([16, N], BF, name=f"a{m}t") for m in range(1, 5)]
            a1r = [cp.tile([128, 16], BF, name=f"a1r{t}") for t in range(NT)]
            a3r = [cp.tile([128, 16], BF, name=f"a3r{t}") for t in range(NT)]
            stgall = cp.tile([80, N], BF, name="stgall")

            env.update(locals())
            if loop:
                _emit_pre(env)
                with tc.For_i(0, reps, 1):
                    _emit_body(env, 0, skip_pre=True)
            else:
                for rep in range(reps):
                    _emit_body(env, rep)
    nc.compile()
    return nc


def _emit_pre(env):
    """Rep-invariant preamble: A-family, hx^T, and Q = (2S^2-I)^T tiles."""
    nc = env["nc"]
    s0t, s1t = env["s0t"], env["s1t"]
    s0q, s1q = env["s0q"], env["s1q"]
    ident, identb, psel = env["ident"], env["identb"], env["psel"]
    hxr, hxT = env["hxr"], env["hxT"]
    a0r, a0t = env["a0r"], env["a0t"]
    amt, a1r, a3r, stgall = env["amt"], env["a1r"], env["a3r"], env["stgall"]
    value = env["value"]
    pdp, ptp, ppp = env["pdp"], env["ptp"], env["ppp"]
    uid = [0]

    def nm(pfx):
        uid[0] += 1
        return f"{pfx}P_{uid[0]}"

    # ---- A-family ^T tiles [16, 1024] + packed stgall ----
    for st, rowt, i1, i2 in ((s0t, a1r, 1, 2), (s1t, a3r, 3, 4)):
        for c2 in range(2):
            cs = slice(c2 * 512, (c2 + 1) * 512)
            pa = ppp.tile([16, 512], F32, name=nm("pa"), tag="pp")
            for kt in range(NT):
                nc.tensor.matmul(pa, a0r[kt], st[kt][:, cs],
                                 start=(kt == 0), stop=(kt == NT - 1))
            nc.any.tensor_copy(amt[i1][:, cs], pa)
        for t in range(NT):
            pr = ptp.tile([128, 16], BF, name=nm("pr"), tag="pt")
            nc.tensor.transpose(pr, amt[i1][:, t * 128:(t + 1) * 128],
                                identb[0:16, 0:16])
            nc.any.tensor_copy(rowt[t], pr)
        for c2 in range(2):
            cs = slice(c2 * 512, (c2 + 1) * 512)
            pa = ppp.tile([16, 512], F32, name=nm("pa2"), tag="pp")
            for kt in range(NT):
                nc.tensor.matmul(pa, rowt[kt], st[kt][:, cs],
                                 start=(kt == 0), stop=(kt == NT - 1))
            nc.vector.scalar_tensor_tensor(
                out=amt[i2][:, cs], in0=pa, scalar=2.0, in1=a0t[:, cs],
                op0=OP.mult, op1=OP.subtract)
    for c2 in range(2):
        cs = slice(c2 * 512, (c2 + 1) * 512)
        pk = ppp.tile([80, 512], F32, name=nm("pk"), tag="pp")
        for m in range(5):
            nc.tensor.matmul(pk, psel[:, m * 80:(m + 1) * 80], amt[m][:, cs],
                             start=(m == 0), stop=(m == 4))
        nc.any.tensor_copy(stgall[:, cs], pk)

    # ---- hx^T ----
    for p in range(4):
        for half in range(2):
            pt = ptp.tile([128, 512], BF, name=nm("px"), tag="pt")
            for ti in range(4):
                t = half * 4 + ti
                nc.tensor.transpose(pt[:, ti * 128:(ti + 1) * 128],
                                    hxr[t][:, p * 128:(p + 1) * 128], ident)
            nc.vector.tensor_copy(hxT[p][:, half * 512:(half + 1) * 512], pt)

    # ---- Q_i = 2*(S_i^T S_i^T) - I, using value tiles as S-node scratch ----
    for st, sq in ((s0t, s0q), (s1t, s1q)):
        for t in range(NT):
            for half in range(2):
                pt = ptp.tile([128, 512], BF, name=nm("sn"), tag="pt")
                for ki in range(4):
                    kt = half * 4 + ki
                    nc.tensor.transpose(pt[:, ki * 128:(ki + 1) * 128],
                                        st[kt][:, t * 128:(t + 1) * 128], ident)
                nc.vector.tensor_copy(
                    value[t][:, half * 512:(half + 1) * 512], pt)
        for a in range(NT):
            for half in range(2):
                cs = slice(half * 512, (half + 1) * 512)
                pq = pdp.tile([128, 512], F32, name=nm("pq"), tag="pd")
                for mt in range(NT):
                    nc.tensor.matmul(pq, value[mt][:, a * 128:(a + 1) * 128],
                                     st[mt][:, cs],
                                     start=(mt == 0), stop=(mt == NT - 1))
                nc.vector.tensor_scalar_mul(sq[a][:, cs], pq, 2.0)
            dcs = slice(a * 128, (a + 1) * 128)
            nc.vector.tensor_sub(sq[a][:, dcs], sq[a][:, dcs], ident)


def _emit_body(env, rep, skip_pre=False):
    nc = env["nc"]
    s0t, s1t = env["s0t"], env["s1t"]
    s0q, s1q = env["s0q"], env["s1q"]
    wo_g, wu_g = env["wo_g"], env["wu_g"]
    waoj, wauj = env["waoj"], env["wauj"]
    b_o, b_u = env["b_o"], env["b_u"]
    ident, identb = env["ident"], env["identb"]
    hxr, hxT = env["hxr"], env["hxT"]
    h0b = env["h0b"]
    hallT, value = env["hallT"], env["value"]
    un, cn = env["un"], env["cn"]
    op_ = env["op_"]
    pdp, ptp, ppp = env["pdp"], env["ptp"], env["ppp"]
    out_d = env["out_d"]
    R = f"r{rep}"
    uid = [0]

    def nm(pfx):
        uid[0] += 1
        return f"{pfx}{R}_{uid[0]}"

    def apply_p(s, x, M, p):
        # hallT[s][p] = (x^T @ M) chunk p.  kt-outer over the two 512-col
        # halves: consecutive matmuls share the same stationary x[kt] chunk.
        pd = [pdp.tile([128, 512], F32, name=nm("pd"), tag="pd")
              for _ in range(2)]
        for kt in range(NT):
            for half in range(2):
                cs = slice(half * 512, (half + 1) * 512)
                nc.tensor.matmul(pd[half], x[kt][:, p * 128:(p + 1) * 128],
                                 M[kt][:, cs],
                                 start=(kt == 0), stop=(kt == NT - 1))
        for half in range(2):
            cs = slice(half * 512, (half + 1) * 512)
            nc.vector.tensor_copy(hallT[s][p][:, cs], pd[half])

    def t_slot(s, src):
        # hallT[s] = src^T  (node [1024, 512] -> feature [512, 1024])
        for p in range(4):
            for half in range(2):
                pt = ptp.tile([128, 512], BF, name=nm("pt"), tag="pt")
                for ti in range(4):
                    t = half * 4 + ti
                    nc.tensor.transpose(pt[:, ti * 128:(ti + 1) * 128],
                                        src[t][:, p * 128:(p + 1) * 128], ident)
                nc.vector.tensor_copy(
                    hallT[s][p][:, half * 512:(half + 1) * 512], pt)

    def t_val(rows, dst, mul_by=None):
        # dst[t][:, j*64:(j+1)*64] = value[j][rows].T  (64-row transposes);
        # with mul_by, dst[t] = transpose * mul_by[t] fused from PSUM.
        lo = rows.start
        for t in range(NT):
            pt = ptp.tile([128, 512], BF, name=nm("pv"), tag="pt")
            for j in range(J):
                nc.tensor.transpose(pt[:, j * 64:(j + 1) * 64],
                                    value[j][rows, t * 128:(t + 1) * 128],
                                    identb[lo:lo + 64, :])
            if mul_by is None:
                nc.vector.tensor_copy(dst[t], pt)
            else:
                nc.vector.tensor_mul(dst[t], pt, mul_by[t])

    def project_p(gc, p):
        wg, wa, ob, bias = ((wo_g, waoj, O1, b_o) if gc == 0
                            else (wu_g, wauj, O2, b_u))
        stgall = env["stgall"]
        for j in (2 * p, 2 * p + 1):
            jo = (j % 2) * 64
            for c2 in range(2):
                cs = slice(c2 * 512, (c2 + 1) * 512)
                pp = ppp.tile([ob, 512], F32, name=nm("pp"), tag="pp")
                for i in range(5):
                    rhs = (hxT[p] if (gc == 0 and i == 0)
                           else hallT[i][p] if i > 0 else hallT[0][p])
                    nc.tensor.matmul(pp, wg[i][jo:jo + 64, :],
                                     rhs[jo:jo + 64, cs],
                                     start=(i == 0), stop=False)
                nc.tensor.matmul(pp, wa[:, j * ob:(j + 1) * ob],
                                 stgall[:, cs], start=False, stop=True)
                if gc == 0:
                    nc.scalar.activation(out=value[j][:, cs], in_=pp,
                                         func=AF.Sigmoid, bias=bias, scale=1.0)
                else:
                    nc.scalar.activation(out=value[j][0:O2, cs], in_=pp,
                                         func=AF.Tanh, bias=bias, scale=1.0)

    SLOTS = ((1, s0t), (2, s0q), (3, s1t), (4, s1q))

    # ================= gconv 1 =================
    if STAGE < 1:
        return
    if rep == 0 and not skip_pre:
        _emit_pre(env)
    # p-block pipelining: projection of tile-pair p overlaps applies of p+1
    for p in range(4):
        for s, M in SLOTS:
            apply_p(s, hxr, M, p)
        if STAGE >= 3:
            project_p(0, p)
    if STAGE < 4:
        return

    # x'0 = r^T-transpose * hx (fused from PSUM); u -> node domain
    t_val(slice(0, 64), h0b, mul_by=hxr)
    t_val(slice(64, 128), un)
    if STAGE < 5:
        return

    # ================= gconv 2 =================
    t_slot(0, h0b)
    for p in range(4):
        for s, M in SLOTS:
            apply_p(s, h0b, M, p)
        if STAGE >= 6:
            project_p(1, p)
    if STAGE < 7:
        return
    t_val(slice(0, 64), cn)
    if STAGE < 8:
        return

    # ---- final blend in node domain: out = c + u*(hx - c) ----
    for t in range(NT):
        orow = op_.tile([128, J * U], F32, name=nm("or"), tag="orow")
        nc.vector.tensor_sub(orow, hxr[t], cn[t])
        nc.vector.tensor_mul(orow, un[t], orow)
        nc.vector.tensor_add(orow, cn[t], orow)
        if STAGE < 9:
            continue
        nc.sync.dma_start(
            out_d.rearrange("j (n u) -> n j u", u=U)[t * 128:(t + 1) * 128],
            orow.rearrange("p (j u) -> p j u", j=J),
        )


def _prep_shared(weights_output, biases_output, weights_update, biases_update):
    bf = ml_dtypes.bfloat16
    maps = {}
    for tag, W, ob in (("o", weights_output, O1), ("u", weights_update, O2)):
        Wr = W.reshape(66, 5, ob)
        H = Wr[2:, :, :]
        A = Wr[:2, :, :]
        for i in range(5):
            blk = np.concatenate([H[:, i], H[:, i]])   # rows duplicated at 0/64
            maps[f"w{tag}_g{i}"] = np.ascontiguousarray(blk).astype(bf)
        waj = np.zeros((80, J * ob), np.float32)
        for j in range(J):
            for m in range(5):
                for f in range(2):
                    waj[m * 16 + 2 * j + f, j * ob:(j + 1) * ob] = A[f, m]
        maps["waoj" if tag == "o" else "wauj"] = waj.astype(bf)
    maps["b_o"] = np.ascontiguousarray(biases_output.astype(np.float32)[:, None])
    maps["b_u"] = np.ascontiguousarray(biases_update.astype(np.float32)[:, None])
    maps["ident"] = np.eye(128, dtype=np.float32).astype(bf)
    maps["identb"] = np.concatenate([np.eye(64), np.eye(64)]).astype(bf)
    psel = np.zeros((16, 400), np.float32)
    for m in range(5):
        for i in range(16):
            psel[i, m * 80 + m * 16 + i] = 1.0
    maps["psel"] = psel.astype(bf)
    return maps


def make_in_maps(inputs, hx, support0, support1, weights_output, biases_output,
                 weights_update, biases_update):
    bf = ml_dtypes.bfloat16
    shared = _prep_shared(np.asarray(weights_output, dtype=np.float32),
                          np.asarray(biases_output, dtype=np.float32),
                          np.asarray(weights_update, dtype=np.float32),
                          np.asarray(biases_update, dtype=np.float32))
    shared["s0t"] = np.ascontiguousarray(np.asarray(support0, np.float32).T).astype(bf)
    shared["s1t"] = np.ascontiguousarray(np.asarray(support1, np.float32).T).astype(bf)

    hx = np.asarray(hx, dtype=np.float32)
    xi = np.asarray(inputs, dtype=np.float32).reshape(B, N, D_IN)
    hx3 = hx.reshape(B, N, U)

    in_maps = []
    for c in range(NCORES):
        sl = slice(c * J, (c + 1) * J)
        hxc = hx3[sl].transpose(1, 0, 2).reshape(N, J * U)
        a0 = xi[sl].transpose(1, 0, 2).reshape(N, 16)   # [n, (j,f)]
        m = dict(shared)
        m["hxr"] = hxc.astype(bf)
        m["a0r"] = a0.astype(bf)
        m["a0t"] = np.ascontiguousarray(a0.T).astype(bf)
        in_maps.append(m)
    return in_maps


def kernel(inputs, hx, support0, support1, weights_output, biases_output,
           weights_update, biases_update):
    if "nc" not in _CACHE:
        _CACHE["nc"] = _build()
    nc = _CACHE["nc"]
    in_maps = make_in_maps(inputs, hx, support0, support1, weights_output,
                           biases_output, weights_update, biases_update)
    res = run_bass_kernel_spmd(nc, in_maps, core_ids=list(range(NCORES)))
    return np.concatenate([r["out"] for r in res.results], axis=0)
